# revision 1
# baseline (speedup 1.0000x reference)
"""Trainium2 Bass kernel for nn_DecoderForLarge (sparse attention decoder).

Shapes (hardcoded): B=64, N=1000, G=500, H=256. 8 NeuronCores, batch-sharded
(8 batches per core). All matmuls run as float32r (12-bit mantissa) with
hi/lo split-precision compensation:
  - pooled  = maskprob @ emb:   mask side exact (values 0 / -2^26, stored
              bf16, sign folded into W_visited), emb split hi/lo -> 2 terms,
              fp32 PSUM accumulate.
  - q/score = W @ x, fq @ embT: both sides split, 3 terms (hi*hi+hi*lo+lo*hi).
The -2^26 additive mask is applied *before* the tanh clip: tanh saturates to
-1 so visited nodes land at clip=-10, and exp(-10 - rowmax) with rowmax ~ +10
underflows to ~1e-9 relative - equivalent to the reference's hard -1e8 mask
for this data regime.
Error vs fp32 reference ~5e-4 absmax-relative (validated on HW).
"""

import sys

for _p in ("/opt/trn_rl_repo", "/root/.axon_site/_ro/trn_rl_repo"):
    if _p not in sys.path:
        sys.path.append(_p)

import numpy as np

import concourse.bass as bass
import concourse.mybir as mybir
import concourse.tile as tile
from concourse.masks import make_identity
from concourse.bass_utils import run_bass_kernel_spmd

F32 = mybir.dt.float32
F32R = mybir.dt.float32r
BF16 = mybir.dt.bfloat16
I32 = mybir.dt.int32

B, N, G, H = 64, 1000, 500, 256
NCORES = 8
NB = B // NCORES          # batches per core
GC = 125                  # G chunk (4 chunks of 125)
NGC = G // GC
NCH = 8                   # N partition chunks (7 x 128 + 104)
NLAST = N - 7 * 128       # 104
TANH_CLIP = 10.0
INV_SQRT_H = float(1.0 / np.sqrt(np.float32(H)))
NEG_INV_SQRT_2 = float(-np.float32(1.0 / np.sqrt(2.0)))
MASK_NEG = -float(2.0 ** 26)   # additive mask; exact in bf16/f32r


def _r12(x):
    """Round to nearest with 12-bit mantissa (f32r representable values)."""
    x = np.ascontiguousarray(x, np.float32)
    u = x.view(np.uint32).astype(np.uint64)
    shift = 23 - 12
    u = ((u + (1 << (shift - 1))) >> shift) << shift
    return (u & np.uint64(0xFFFFFFFF)).astype(np.uint32).view(np.float32)


def _split_hilo(x):
    hi = _r12(x)
    lo = _r12(np.float32(x) - hi)
    return hi, lo


def _nsz(c):
    """partition size of N-chunk c"""
    return 128 if c < 7 else NLAST


def _split_excess_waits(nc, maxw=1):
    # This walrus build rejects >1 semaphore wait per instruction
    # (CoreV3 setupSyncWait). Move extras onto preceding same-engine NoOps.
    for f in nc.m.functions:
        for bb in f.blocks:
            newlist = []
            for ins in bb.instructions:
                si = ins.sync_info
                if si is not None and si.on_wait is not None and len(si.on_wait) > maxw:
                    waits = list(si.on_wait)
                    extra, keep = waits[:-maxw], waits[-maxw:]
                    for i in range(0, len(extra), maxw):
                        nop = mybir.InstNoOp(name=f"{ins.name}-ws{i}", ins=[], outs=[])
                        nop.engine = ins.engine
                        nop.sync_info = mybir.SyncInfo(on_wait=extra[i:i + maxw], on_update=[])
                        newlist.append(nop)
                    ins.sync_info = mybir.SyncInfo(on_wait=keep, on_update=list(si.on_update or []))
                newlist.append(ins)
            bb.instructions[:] = newlist


def build_nc(nb=NB, debug=False, ablate=()):
    nc = bass.Bass("TRN2", target_bir_lowering=False, debug=False,
                   num_swdge_queues=4)
    Alu = mybir.AluOpType
    Act = mybir.ActivationFunctionType

    def _on_queue(inst, qn):
        # indirect_dma_start pins queue="qPoolDynamic"; rotate across the 4
        # SWDGE queues to spread descriptors over more SDMA engines
        if qn:
            inst.ins.queue = f"qPoolDynamic{qn}"
        return inst

    dbg = {}
    if debug:
        for nm, shp in (("d_maskT", [128, NCH, G + 4]), ("d_pooled_hi", [128, 2, G + 1]),
                        ("d_pooled_lo", [128, 2, G + 1]), ("d_fq_hi", [128, 2, G]),
                        ("d_fq_lo", [128, 2, G]), ("d_embT_hi", [128, 2, N]),
                        ("d_lastT_hi", [128, 2, G]), ("d_z", [GC, N]),
                        ("d_z3", [GC, N]), ("d_qg", [1, H])):
            dbg[nm] = nc.dram_tensor(nm, shp, F32, kind="ExternalOutput").ap()

    emb_e = nc.dram_tensor("emb", [nb, N, H], F32, kind="ExternalInput").ap()
    dist_e = nc.dram_tensor("dists", [nb, N, N], F32, kind="ExternalInput").ap()
    ln_e = nc.dram_tensor("last_node", [nb, G], I32, kind="ExternalInput").ap()
    mask_e = nc.dram_tensor("mask", [nb, G, N], F32, kind="ExternalInput").ap()
    w_e = {}
    for w in ("wlf_hi", "wlf_lo", "wv_hi", "wv_lo", "wg_hi", "wg_lo"):
        w_e[w] = nc.dram_tensor(w, [H, H], F32R, kind="ExternalInput").ap()
    out_e = nc.dram_tensor("out", [nb, G, N], F32, kind="ExternalOutput").ap()

    emb_flat = emb_e.rearrange("b n h -> (b n) h")
    dist_flat = dist_e.rearrange("b n m -> (b n) m")

    with tile.TileContext(nc) as tc:
        import contextlib
        with contextlib.ExitStack() as ctx:
            const = ctx.enter_context(tc.tile_pool(name="const", bufs=1))
            io1 = ctx.enter_context(tc.tile_pool(name="io1", bufs=1))
            der = ctx.enter_context(tc.tile_pool(name="der", bufs=1))
            der2 = ctx.enter_context(tc.tile_pool(name="der2", bufs=2))
            distp = ctx.enter_context(tc.tile_pool(name="distp", bufs=4))
            dmaskp = ctx.enter_context(tc.tile_pool(name="dmaskp", bufs=3))
            sm = ctx.enter_context(tc.tile_pool(name="sm", bufs=2))
            tiny = ctx.enter_context(tc.tile_pool(name="tiny", bufs=4))
            ps_tp = ctx.enter_context(tc.tile_pool(name="ps_tp", bufs=2, space="PSUM"))
            ps_pq = ctx.enter_context(tc.tile_pool(name="ps_pq", bufs=2, space="PSUM"))
            ps_sc = ctx.enter_context(tc.tile_pool(name="ps_sc", bufs=4, space="PSUM"))

            # ---- constants ----
            identf = const.tile([128, 128], F32, name="identf")
            make_identity(nc, identf[:])
            identr = const.tile([128, 128], F32R, name="identr")
            nc.vector.tensor_copy(out=identr[:], in_=identf[:])
            identb = const.tile([128, 128], BF16, name="identb")
            nc.vector.tensor_copy(out=identb[:], in_=identf[:])
            ones_f = const.tile([128, 4], F32, name="ones_f")
            nc.gpsimd.memset(ones_f[:], 1.0)
            ones_row = const.tile([1, G], F32R, name="ones_row")
            nc.vector.tensor_copy(out=ones_row[:], in_=ones_f[0:1, 0:1].to_broadcast([1, G]))
            wt = {}
            for w, ap_ in w_e.items():
                t = const.tile([128, 2, H], F32R, name=w)
                nc.sync.dma_start(out=t[:], in_=ap_.rearrange("(c p) o -> p c o", p=128))
                wt[w] = t

            TERMS = (("hi", "hi"),) if "nolo" in ablate else \
                (("hi", "hi"), ("lo", "hi"), ("hi", "lo"))

            def head(b):
                st = {}
                # ---- indices ----
                idx = tiny.tile([GC, NGC], I32, name="idx")
                nc.sync.dma_start(
                    out=idx[:],
                    in_=ln_e[b].rearrange("(c p) -> p c", p=GC))
                idxg = tiny.tile([GC, NGC], I32, name="idxg")
                nc.vector.tensor_scalar_add(idxg[:], idx[:], b * N)

                # ---- gathers (early: let SDMA work in background) ----
                lastemb = der.tile([GC, NGC, H], F32, name="lastemb")
                for gc in range(NGC):
                    _on_queue(nc.gpsimd.indirect_dma_start(
                        out=lastemb[:, gc, :], out_offset=None, in_=emb_flat,
                        in_offset=bass.IndirectOffsetOnAxis(ap=idxg[:, gc:gc + 1], axis=0)),
                        gc)
                dist_t = []
                for gc in range(NGC):
                    dt_ = distp.tile([GC, N], F32, name="dist")
                    if "nodma" not in ablate:
                        _on_queue(nc.gpsimd.indirect_dma_start(
                            out=dt_[:], out_offset=None, in_=dist_flat,
                            in_offset=bass.IndirectOffsetOnAxis(ap=idxg[:, gc:gc + 1], axis=0)),
                            gc)
                    else:
                        nc.vector.memset(dt_[:, 0:2], 0.0)
                    dist_t.append(dt_)

                # ---- mask: load + single conversion to {0, -2^26} (bf16) ----
                mraw = io1.tile([GC, NGC, N], F32, name="mraw")
                if "nodma" not in ablate:
                    _on_queue(nc.gpsimd.dma_start(
                        out=mraw[:], in_=mask_e[b].rearrange("(c p) n -> p c n", p=GC)),
                        b % 4)
                else:
                    nc.vector.memset(mraw[:, 0, 0:2], 0.0)
                maskprob = der.tile([GC, NGC, N], BF16, name="maskprob")
                nc.vector.tensor_scalar_max(maskprob[:], mraw[:], MASK_NEG)

                # ---- dmask[gc] = dist * (-1/sqrt2) + maskprob  (off the tail) ----
                dmask_t = []
                for gc in range(NGC):
                    dm = dmaskp.tile([GC, N], F32, name="dmask")
                    nc.vector.scalar_tensor_tensor(
                        out=dm[:], in0=dist_t[gc][:], scalar=NEG_INV_SQRT_2,
                        in1=maskprob[:, gc, :], op0=Alu.mult, op1=Alu.add)
                    dmask_t.append(dm)

                # ---- embeddings: load + hi/lo split. Interleaved chunking
                # n = 8p + c gives 8KB-contiguous per-partition DMA extents
                # (125 descriptors instead of 1000); row 125 zero-padded so
                # matmul K=126 stays even (f32r requirement)
                emb_f = io1.tile([128, NCH, H], F32, name="emb_f")
                # zero the 96:128 block first (partition slices must be
                # 32-aligned); the DMA then overwrites rows 96:125, leaving
                # pad row 125 zero so matmul K=126 stays even (f32r rule)
                nc.vector.memset(emb_f[96:128, :, :], 0.0)
                nc.sync.dma_start(
                    out=emb_f[0:GC, :, :],
                    in_=emb_e[b].rearrange("(p c) h -> p c h", c=NCH))
                emb_hi = der.tile([128, NCH, H], F32R, name="emb_hi")
                nc.vector.tensor_copy(out=emb_hi[:], in_=emb_f[:])
                emb_lo = der.tile([128, NCH, H], F32R, name="emb_lo")
                nc.vector.scalar_tensor_tensor(
                    out=emb_lo[:], in0=emb_hi[:], scalar=-1.0, in1=emb_f[:],
                    op0=Alu.mult, op1=Alu.add)

                # ---- maskT: PE-transpose maskprob (bf16, FWL weight loads);
                # 4 g-blocks share one PSUM tile (126-col pitch), then a single
                # strided copy per n-chunk. cols G..G+3 = 1.0 (mean pooling) ----
                maskT = der.tile([128, NCH, G + 4], F32R, name="maskT")
                # f32r memset fails the ISA checker; memset the f32 bit-view
                nc.vector.memset(maskT[96:128, :, :].bitcast(F32), 0.0)
                for c in range(NCH):
                    ptp = ps_tp.tile([128, 504], BF16, name="tpb", tag="tp")
                    mp_il = maskprob[:, :, :].rearrange("p a (q c) -> p a c q", c=NCH)
                    for gc in range(NGC):
                        nc.tensor.matmul(
                            out=ptp[:GC, gc * 126:(gc + 1) * 126],
                            lhsT=mp_il[:, gc, c, :],
                            rhs=identb[:GC, :126],
                            is_transpose=True, skip_group_check=True)
                    nc.scalar.copy(
                        out=maskT[:GC, c, 0:G].rearrange("p (a g) -> p a g", a=NGC),
                        in_=ptp[:GC, :].rearrange("p (a g) -> p a g", a=NGC)[:, :, 0:GC])
                    nc.vector.tensor_copy(
                        out=maskT[:GC, c, G:G + 4], in_=ones_f[:GC, :])
                    # pad row 125 keeps zeros (memset above) for the mean col

                # ---- embT: transpose hi/lo (f32r); 4 n-blocks per PSUM tile ----
                embT = {}
                for t_, esrc in (("hi", emb_hi), ("lo", emb_lo)):
                    dstt = der2.tile([128, 2, N], F32R, name=f"embT_{t_}")
                    for hc in range(2):
                        for half in range(2):
                            cs = range(4 * half, 4 * half + 4)
                            ptp = ps_tp.tile([128, 512], F32R, name="tpr", tag="tp")
                            for j, c in enumerate(cs):
                                # 128-col pitch to match the (a q), q=128 view below
                                nc.tensor.matmul(
                                    out=ptp[:, j * 128:j * 128 + 126],
                                    lhsT=esrc[:GC, c, hc * 128:(hc + 1) * 128],
                                    rhs=identr[:GC, :126],
                                    is_transpose=True, skip_group_check=True)
                            # scatter block columns back to natural n order
                            # (n = 8q + c): out free dims (c, q) strides (1, 8)
                            ov = dstt[:, hc, :].rearrange("p (q c) -> p c q", c=NCH)
                            nc.scalar.copy(
                                out=ov[:, 4 * half:4 * half + 4, :],
                                in_=ptp[:, :].rearrange("p (a q) -> p a q", a=4)[:, :, 0:GC])
                    embT[t_] = dstt

                # ---- pooled^T (+ mean cols) ----
                pooled = {"hi": der.tile([128, 2, G + 1], F32R, name="pooled_hi"),
                          "lo": der.tile([128, 2, G + 1], F32R, name="pooled_lo")}
                pl_ps = []
                terms_p = ("hi",) if "nolo" in ablate else ("hi", "lo")
                for hc in range(2):
                    pp = ps_pq.tile([128, G + 4], F32, name="pp", tag="pq")
                    mms = [(t_, c) for t_ in terms_p for c in range(NCH)]
                    for i, (t_, c) in enumerate(mms):
                        esrc = emb_hi if t_ == "hi" else emb_lo
                        nc.tensor.matmul(
                            out=pp[:, :G + 4],
                            lhsT=esrc[:126, c, hc * 128:(hc + 1) * 128],
                            rhs=maskT[:126, c, :],
                            start=(i == 0), stop=(i == len(mms) - 1))
                    pl_ps.append(pp)
                for hc in range(2):
                    nc.vector.tensor_copy(out=pooled["hi"][:, hc, :], in_=pl_ps[hc][:, :G + 1])
                    nc.vector.scalar_tensor_tensor(
                        out=pooled["lo"][:, hc, :], in0=pooled["hi"][:, hc, :],
                        scalar=-1.0, in1=pl_ps[hc][:, :G + 1], op0=Alu.mult, op1=Alu.add)

                # ---- last_emb^T: transpose f32 (4 g-blocks per PSUM tile), split ----
                lastT = {"hi": der.tile([128, 2, G], F32R, name="lastT_hi"),
                         "lo": der.tile([128, 2, G], F32R, name="lastT_lo")}
                for hc in range(2):
                    ptp = ps_tp.tile([128, 504], F32, name="tpf", tag="tp")
                    for gc in range(NGC):
                        nc.tensor.matmul(
                            out=ptp[:, gc * 126:gc * 126 + GC],
                            lhsT=lastemb[:, gc, hc * 128:(hc + 1) * 128],
                            rhs=identf[:GC, :GC],
                            is_transpose=True, skip_group_check=True)
                    pv = ptp[:, :].rearrange("p (a g) -> p a g", a=NGC)[:, :, 0:GC]
                    hv = lastT["hi"][:, hc, :].rearrange("p (a g) -> p a g", a=NGC)
                    nc.vector.tensor_copy(out=hv, in_=pv)
                    nc.vector.scalar_tensor_tensor(
                        out=lastT["lo"][:, hc, :].rearrange("p (a g) -> p a g", a=NGC),
                        in0=hv, scalar=-1.0, in1=pv, op0=Alu.mult, op1=Alu.add)

                # ---- q_graph^T row: qg[1, H] = sum over terms mc_x.T @ Wg_y ----
                qg_ps = ps_pq.tile([1, H], F32, name="qg", tag="pq")
                i = 0
                for (tx, tw) in TERMS:
                    for kc in range(2):
                        nc.tensor.matmul(
                            out=qg_ps[:, :],
                            lhsT=pooled[tx][:, kc, G:G + 1],
                            rhs=wt[f"wg_{tw}"][:, kc, :],
                            start=(i == 0), stop=(i == len(TERMS) * 2 - 1))
                        i += 1
                qg_row = tiny.tile([1, H], F32R, name="qg_row")
                nc.vector.tensor_copy(out=qg_row[:], in_=qg_ps[:, :])

                # ---- fq^T = q_lf + q_vis + qg (rank-1 broadcast matmul) ----
                fq = {"hi": der2.tile([128, 2, G], F32R, name="fq_hi"),
                      "lo": der2.tile([128, 2, G], F32R, name="fq_lo")}
                for hc in range(2):
                    qp = ps_pq.tile([128, G], F32, name="qp", tag="pq")
                    mms = []
                    for (tx, tw) in TERMS:
                        for kc in range(2):
                            mms.append((lastT[tx][:, kc, :], wt[f"wlf_{tw}"][:, kc, hc * 128:(hc + 1) * 128]))
                    for (tx, tw) in TERMS:
                        for kc in range(2):
                            mms.append((pooled[tx][:, kc, 0:G], wt[f"wv_{tw}"][:, kc, hc * 128:(hc + 1) * 128]))
                    # qg broadcast over g: rank-1 matmul, K=1
                    mms.append((ones_row[:, :], qg_row[:1, hc * 128:(hc + 1) * 128]))
                    for i, (xap, wap) in enumerate(mms):
                        nc.tensor.matmul(
                            out=qp[:, :G], lhsT=wap, rhs=xap,
                            start=(i == 0), stop=(i == len(mms) - 1))
                    nc.vector.tensor_copy(out=fq["hi"][:, hc, :], in_=qp[:, :G])
                    nc.vector.scalar_tensor_tensor(
                        out=fq["lo"][:, hc, :], in0=fq["hi"][:, hc, :],
                        scalar=-1.0, in1=qp[:, :G],
                        op0=Alu.mult, op1=Alu.add)

                st.update(fq=fq, embT=embT, dmask_t=dmask_t, maskT=maskT,
                          pooled=pooled, lastT=lastT, qg_row=qg_row)
                return st

            def tail(b, st):
                fq, embT, dmask_t = st["fq"], st["embT"], st["dmask_t"]
                # ---- score + softmax per g-chunk ----
                for gc in range(NGC):
                    # one PSUM tile per 500-col half: a matmul output must stay
                    # inside a single 2KB PSUM bank
                    sc = [ps_sc.tile([GC, 500], F32, name="sc", tag="sc")
                          for _ in range(2)]
                    nmm = len(TERMS) * 2
                    for nh in range(2):
                        i = 0
                        for (tf, te) in TERMS:
                            for kc in range(2):
                                nc.tensor.matmul(
                                    out=sc[nh][:, :],
                                    lhsT=fq[tf][:, kc, gc * GC:(gc + 1) * GC],
                                    rhs=embT[te][:, kc, nh * 500:(nh + 1) * 500],
                                    start=(i == 0), stop=(i == nmm - 1))
                                i += 1
                    # z = score + (mask - dist/sqrt2); tanh saturation applies
                    # the -2^26 mask (visited -> clip exactly -10)
                    z = sm.tile([GC, N], F32, name="z")
                    for nh in range(2):
                        nc.vector.tensor_tensor(
                            out=z[:, nh * 500:(nh + 1) * 500],
                            in0=dmask_t[gc][:, nh * 500:(nh + 1) * 500],
                            in1=sc[nh][:, :], op=Alu.add)
                    t_ = sm.tile([GC, N], F32, name="t")
                    nc.scalar.activation(out=t_[:], in_=z[:], func=Act.Tanh, scale=1.0)
                    nm = tiny.tile([GC, 1], F32, name="nm")
                    nc.vector.tensor_reduce(
                        out=nm[:], in_=t_[:], axis=mybir.AxisListType.X,
                        op=Alu.max, negate=True)
                    nm10 = tiny.tile([GC, 1], F32, name="nm10")
                    nc.vector.tensor_scalar_mul(nm10[:], nm[:], TANH_CLIP)
                    e = z                                  # write exp in place
                    s = tiny.tile([GC, 1], F32, name="s")
                    nc.scalar.activation(
                        out=e[:], in_=t_[:], func=Act.Exp, bias=nm10[:, :1],
                        scale=TANH_CLIP, accum_out=s[:, :1])
                    r = tiny.tile([GC, 1], F32, name="r")
                    nc.vector.reciprocal(out=r[:], in_=s[:, :1])
                    o = t_                                 # write output in place
                    nc.scalar.activation(out=o[:], in_=e[:], func=Act.Copy,
                                         scale=r[:, :1])
                    _on_queue(nc.gpsimd.dma_start(
                        out=out_e[b, gc * GC:(gc + 1) * GC, :], in_=o[:]),
                        (gc + 2) % 4)
                    if debug and b == 0 and gc == 0:
                        nc.sync.dma_start(out=dbg["d_z"][:], in_=z[:])
                        nc.sync.dma_start(out=dbg["d_z3"][:], in_=t_[:])

                if debug and b == 0:
                    for nm_, t in (("d_maskT", st["maskT"]), ("d_embT_hi", embT["hi"]),
                                   ("d_lastT_hi", st["lastT"]["hi"]), ("d_qg", st["qg_row"]),
                                   ("d_pooled_hi", st["pooled"]["hi"]), ("d_pooled_lo", st["pooled"]["lo"]),
                                   ("d_fq_hi", fq["hi"]), ("d_fq_lo", fq["lo"])):
                        nc.gpsimd.dma_start(out=dbg[nm_][:], in_=t[:])

            # 1-batch software pipeline: emit head(b+1) before tail(b) so each
            # engine's in-order queue interleaves next-batch prep with the
            # current batch's score/softmax tail
            st = head(0)
            for b in range(nb):
                st_next = head(b + 1) if b + 1 < nb else None
                tail(b, st)
                st = st_next

    _split_excess_waits(nc)
    return nc


_NC_CACHE = {}


def _get_nc(nb=NB):
    if nb not in _NC_CACHE:
        _NC_CACHE[nb] = build_nc(nb)
    return _NC_CACHE[nb]


def _prep_weights(Wq_graph, Wq_first, Wq_last, W_visited):
    Wq_graph = np.asarray(Wq_graph, np.float32)
    Wq_first = np.asarray(Wq_first, np.float32)
    Wq_last = np.asarray(Wq_last, np.float32)
    W_visited = np.asarray(W_visited, np.float32)
    s_h = np.float32(INV_SQRT_H)
    wlf = ((Wq_last + Wq_first).T * s_h).astype(np.float32)
    # maskprob is -2^26 * visited; fold the sign and scale into W_visited
    wv = (W_visited.T * (-s_h / np.float32(N * (-MASK_NEG)))).astype(np.float32)
    wg = (Wq_graph.T * (s_h / np.float32(N))).astype(np.float32)
    out = {}
    out["wlf_hi"], out["wlf_lo"] = _split_hilo(wlf)
    out["wv_hi"], out["wv_lo"] = _split_hilo(wv)
    out["wg_hi"], out["wg_lo"] = _split_hilo(wg)
    return out


def kernel(embeddings, dists, last_node, group_ninf_mask,
           Wq_graph, Wq_first, Wq_last, W_visited, **_ignored):
    embeddings = np.ascontiguousarray(np.asarray(embeddings), np.float32)
    dists = np.ascontiguousarray(np.asarray(dists), np.float32)
    group_ninf_mask = np.ascontiguousarray(np.asarray(group_ninf_mask), np.float32)
    ln = np.ascontiguousarray(np.asarray(last_node)).astype(np.int32)
    w = _prep_weights(Wq_graph, Wq_first, Wq_last, W_visited)

    nc = _get_nc(NB)
    in_maps = []
    for c in range(NCORES):
        sl = slice(c * NB, (c + 1) * NB)
        m = dict(emb=embeddings[sl], dists=dists[sl],
                 last_node=ln[sl], mask=group_ninf_mask[sl])
        m.update(w)
        in_maps.append(m)
    res = run_bass_kernel_spmd(nc, in_maps, list(range(NCORES)))
    out = np.concatenate([res.results[c]["out"] for c in range(NCORES)], axis=0)
    return out.astype(np.float32)


if __name__ == "__main__":
    # quick smoke test with random data
    rng = np.random.default_rng(0)
    emb = rng.standard_normal((B, N, H), dtype=np.float32)
    d = rng.random((B, N, N), dtype=np.float32)
    lnod = rng.integers(0, N, (B, G)).astype(np.int32)
    visited = rng.random((B, G, N)) < 0.3
    mask = np.where(visited, -np.inf, 0.0).astype(np.float32)
    s = 1.0 / np.sqrt(H)
    ws = [rng.standard_normal((H, H), dtype=np.float32) * s for _ in range(4)]
    o = kernel(emb, d, lnod, mask, *ws)
    print("out", o.shape, o.dtype, o.sum())



# revision 4
# speedup vs baseline: 234.0496x; 234.0496x over previous
"""Trainium2 Bass kernel for nn_DecoderForLarge (sparse attention decoder).

Shapes (hardcoded): B=64, N=1000, G=500, H=256. 8 NeuronCores, batch-sharded
(8 batches per core).

v3 design:
  - Single-term f32r matmuls (12-bit mantissa) for the q/score path, bf16 for
    the pooled path, bf16 output, fp16 gathered dists. Simulated absmax-rel
    error ~2.1e-3 (dominated by bf16 output), ~10x under the 2e-2 gate.
  - embT (the score rhs) is pre-transposed and f32r-rounded on HOST and DMA'd
    directly; emb is also supplied as bf16 for the pooled lhsT. The f32 emb
    stays in HBM only as the gather source for last-node embeddings.
  - All bulk DMA contiguous per partition and on HWDGE (sync ring for loads,
    scalar ring for the output store). Group dim uses g=4p+c interleave so
    mask loads / out stores are 8KB-per-partition contiguous. Only the
    dist/last-node row gathers are SWDGE (indirect), rotated over 4 queues.
  - maskprob precomputed on host as bf16 {0, -2^26}; applied additively
    pre-tanh (tanh saturates -> visited rows land at clip=-10 and vanish in
    the softmax). Because 10*tanh bounds scores to [-10, 10], the softmax
    needs no row-max subtraction: exp(10*t) directly, with the row sum from
    the activation accumulator.
  - Emission order per batch: head_load(b+1) [DMAs] -> tail(b) [score/softmax]
    -> head_compute(b+1), so tail ops are not queued behind next-batch compute
    on shared engines.
"""

import sys

for _p in ("/opt/trn_rl_repo", "/root/.axon_site/_ro/trn_rl_repo"):
    if _p not in sys.path:
        sys.path.append(_p)

import numpy as np
import ml_dtypes

import concourse.bass as bass
import concourse.mybir as mybir
import concourse.tile as tile
from concourse.masks import make_identity
from concourse.bass_utils import run_bass_kernel_spmd

F32 = mybir.dt.float32
F32R = mybir.dt.float32r
BF16 = mybir.dt.bfloat16
F16 = mybir.dt.float16
I32 = mybir.dt.int32

B, N, G, H = 64, 1000, 500, 256
NCORES = 8
NB = B // NCORES          # batches per core
GC = 125                  # G rows per chunk; g = 4*p + c (p partition, c chunk)
NGC = G // GC             # 4 chunks
NCH = 8                   # N chunks; n = 8*p + c
TANH_CLIP = 10.0
INV_SQRT_H = float(1.0 / np.sqrt(np.float32(H)))
NEG_INV_SQRT_2 = float(-np.float32(1.0 / np.sqrt(2.0)))
MASK_NEG = -float(2.0 ** 26)   # additive mask; exact in bf16


def _r12(x):
    """Round to nearest with 12-bit mantissa (f32r representable values)."""
    x = np.ascontiguousarray(x, np.float32)
    u = x.view(np.uint32).astype(np.uint64)
    shift = 23 - 12
    u = ((u + (1 << (shift - 1))) >> shift) << shift
    return (u & np.uint64(0xFFFFFFFF)).astype(np.uint32).view(np.float32)


def _split_excess_waits(nc, maxw=1):
    # This walrus build rejects >1 semaphore wait per instruction
    # (CoreV3 setupSyncWait). Move extras onto preceding same-engine NoOps.
    for f in nc.m.functions:
        for bb in f.blocks:
            newlist = []
            for ins in bb.instructions:
                si = ins.sync_info
                if si is not None and si.on_wait is not None and len(si.on_wait) > maxw:
                    waits = list(si.on_wait)
                    extra, keep = waits[:-maxw], waits[-maxw:]
                    for i in range(0, len(extra), maxw):
                        nop = mybir.InstNoOp(name=f"{ins.name}-ws{i}", ins=[], outs=[])
                        nop.engine = ins.engine
                        nop.sync_info = mybir.SyncInfo(on_wait=extra[i:i + maxw], on_update=[])
                        newlist.append(nop)
                    ins.sync_info = mybir.SyncInfo(on_wait=keep, on_update=list(si.on_update or []))
                newlist.append(ins)
            bb.instructions[:] = newlist


def build_nc(nb=NB):
    nc = bass.Bass("TRN2", target_bir_lowering=False, debug=False,
                   num_swdge_queues=4)
    Alu = mybir.AluOpType
    Act = mybir.ActivationFunctionType

    def _on_queue(inst, qn):
        # indirect_dma_start pins queue="qPoolDynamic"; rotate across the 4
        # SWDGE queues to spread descriptors over more SDMA engines
        if qn:
            inst.ins.queue = f"qPoolDynamic{qn}"
        return inst

    emb_e = nc.dram_tensor("emb", [nb, N, H], F32, kind="ExternalInput").ap()
    embt_e = nc.dram_tensor("embt", [nb, 128, 2, N], F32R, kind="ExternalInput").ap()
    embbf_e = nc.dram_tensor("embbf", [nb, N, H], BF16, kind="ExternalInput").ap()
    dist_e = nc.dram_tensor("dists", [nb, N, N], F16, kind="ExternalInput").ap()
    ln_e = nc.dram_tensor("last_node", [GC, nb * NGC], I32, kind="ExternalInput").ap()
    mask_e = nc.dram_tensor("mask", [nb, G, N], BF16, kind="ExternalInput").ap()
    w_e = {}
    for w in ("wlf", "wv", "wg"):
        w_e[w] = nc.dram_tensor(w, [128, 2, H], F32R, kind="ExternalInput").ap()
    out_e = nc.dram_tensor("out", [nb, G, N], BF16, kind="ExternalOutput").ap()

    emb_flat = emb_e.rearrange("b n h -> (b n) h")
    dist_flat = dist_e.rearrange("b n m -> (b n) m")

    with tile.TileContext(nc) as tc:
        import contextlib
        with contextlib.ExitStack() as ctx:
            const = ctx.enter_context(tc.tile_pool(name="const", bufs=1))
            io_e = ctx.enter_context(tc.tile_pool(name="io_e", bufs=2))
            io_m = ctx.enter_context(tc.tile_pool(name="io_m", bufs=2))
            io_t = ctx.enter_context(tc.tile_pool(name="io_t", bufs=2))
            der = ctx.enter_context(tc.tile_pool(name="der", bufs=1))
            dm_p = ctx.enter_context(tc.tile_pool(name="dm_p", bufs=2))
            fq_p = ctx.enter_context(tc.tile_pool(name="fq_p", bufs=2))
            distp = ctx.enter_context(tc.tile_pool(name="distp", bufs=8))
            lep = ctx.enter_context(tc.tile_pool(name="lep", bufs=2))
            obp = ctx.enter_context(tc.tile_pool(name="obp", bufs=2))
            sm = ctx.enter_context(tc.tile_pool(name="sm", bufs=2))
            tiny = ctx.enter_context(tc.tile_pool(name="tiny", bufs=4))
            ps_tp = ctx.enter_context(tc.tile_pool(name="ps_tp", bufs=2, space="PSUM"))
            ps_pq = ctx.enter_context(tc.tile_pool(name="ps_pq", bufs=2, space="PSUM"))
            ps_sc = ctx.enter_context(tc.tile_pool(name="ps_sc", bufs=4, space="PSUM"))

            # ---- constants ----
            identf = const.tile([128, 128], F32, name="identf")
            make_identity(nc, identf[:])
            identb = const.tile([128, 128], BF16, name="identb")
            nc.vector.tensor_copy(out=identb[:], in_=identf[:])
            ones_f = const.tile([128, 4], F32, name="ones_f")
            nc.gpsimd.memset(ones_f[:], 1.0)
            ones_row = const.tile([1, G], F32R, name="ones_row")
            nc.vector.tensor_copy(out=ones_row[:], in_=ones_f[0:1, 0:1].to_broadcast([1, G]))
            wt = {}
            for w, ap_ in w_e.items():
                t = const.tile([128, 2, H], F32R, name=w)
                nc.sync.dma_start(out=t[:], in_=ap_)
                wt[w] = t
            idx_all = const.tile([GC, nb * NGC], I32, name="idx_all")
            nc.sync.dma_start(out=idx_all[:], in_=ln_e)

            # maskT: bf16, transposed maskprob + ones cols (mean pooling).
            # Persistent (bufs=1): ones cols written once here; per-batch
            # copies only touch cols 0:G.
            maskT = der.tile([GC, NCH, G + 4], BF16, name="maskT")
            for c in range(NCH):
                nc.vector.tensor_copy(out=maskT[:, c, G:G + 4], in_=ones_f[:GC, :])

            def head_load(b):
                st = {}
                # ---- indices: idxg = idx + b*N (flat row index) ----
                idxg = tiny.tile([GC, NGC], I32, name="idxg")
                nc.vector.tensor_scalar_add(
                    idxg[:], idx_all[:, b * NGC:(b + 1) * NGC], b * N)

                # ---- gathers (early: SDMA works in background) ----
                lastemb = lep.tile([GC, NGC, H], F32, name="lastemb")
                for gc in range(NGC):
                    _on_queue(nc.gpsimd.indirect_dma_start(
                        out=lastemb[:, gc, :], out_offset=None, in_=emb_flat,
                        in_offset=bass.IndirectOffsetOnAxis(ap=idxg[:, gc:gc + 1], axis=0)),
                        gc)
                dist_t = []
                for gc in range(NGC):
                    dt_ = distp.tile([GC, N], F16, name="dist")
                    _on_queue(nc.gpsimd.indirect_dma_start(
                        out=dt_[:], out_offset=None, in_=dist_flat,
                        in_offset=bass.IndirectOffsetOnAxis(ap=idxg[:, gc:gc + 1], axis=0)),
                        gc)
                    dist_t.append(dt_)

                # ---- bulk loads (HWDGE sync ring, contiguous per partition) --
                maskp = io_m.tile([GC, NGC, N], BF16, name="maskp")
                nc.sync.dma_start(
                    out=maskp[:], in_=mask_e[b].rearrange("(p c) n -> p c n", c=NGC))
                embT = io_t.tile([128, 2, N], F32R, name="embT")
                nc.sync.dma_start(out=embT[:], in_=embt_e[b])
                emb_bf = io_e.tile([GC, NCH, H], BF16, name="emb_bf")
                nc.sync.dma_start(
                    out=emb_bf[:], in_=embbf_e[b].rearrange("(p c) h -> p c h", c=NCH))

                st.update(lastemb=lastemb, dist_t=dist_t, maskp=maskp,
                          embT=embT, emb_bf=emb_bf)
                return st

            def head_compute(b, st):
                lastemb, dist_t, maskp = st["lastemb"], st["dist_t"], st["maskp"]
                emb_bf = st["emb_bf"]

                # ---- dmask = dist * (-1/sqrt2) + maskprob ----
                dmask = dm_p.tile([GC, NGC, N], F32, name="dmask")
                for gc in range(NGC):
                    nc.vector.scalar_tensor_tensor(
                        out=dmask[:, gc, :], in0=dist_t[gc][:], scalar=NEG_INV_SQRT_2,
                        in1=maskp[:, gc, :], op0=Alu.mult, op1=Alu.add)

                # ---- maskT: PE-transpose maskprob (bf16); 4 g-blocks share
                # one PSUM tile (126-col pitch), single strided copy per chunk
                # (copies split between scalar and vector engines)
                mp_il = maskp[:, :, :].rearrange("p a (q c) -> p a c q", c=NCH)
                for c in range(NCH):
                    ptp = ps_tp.tile([GC, 504], BF16, name="tpb", tag="tp")
                    for gc in range(NGC):
                        nc.tensor.matmul(
                            out=ptp[:, gc * 126:(gc + 1) * 126],
                            lhsT=mp_il[:, gc, c, :],
                            rhs=identb[:GC, :126],
                            is_transpose=True, skip_group_check=True)
                    src = ptp[:, :].rearrange("p (a g) -> p a g", a=NGC)[:, :, 0:GC]
                    dst = maskT[:, c, 0:G].rearrange("p (a g) -> p a g", a=NGC)
                    if c % 2 == 0:
                        nc.scalar.copy(out=dst, in_=src)
                    else:
                        nc.vector.tensor_copy(out=dst, in_=src)

                # ---- last_emb^T: transpose f32 (4 g-blocks per PSUM tile) ----
                lastT = der.tile([128, 2, G], F32R, name="lastT")
                for hc in range(2):
                    ptp = ps_tp.tile([128, 504], F32, name="tpf", tag="tp")
                    for gc in range(NGC):
                        nc.tensor.matmul(
                            out=ptp[:, gc * 126:gc * 126 + GC],
                            lhsT=lastemb[:, gc, hc * 128:(hc + 1) * 128],
                            rhs=identf[:GC, :GC],
                            is_transpose=True, skip_group_check=True)
                    nc.vector.tensor_copy(
                        out=lastT[:, hc, :].rearrange("p (a g) -> p a g", a=NGC),
                        in_=ptp[:, :].rearrange("p (a g) -> p a g", a=NGC)[:, :, 0:GC])

                # ---- pooled^T (+ mean col): bf16 matmul, K=125 ----
                pooled = der.tile([128, 2, G + 1], F32R, name="pooled")
                for hc in range(2):
                    pp = ps_pq.tile([128, G + 4], F32, name="pp", tag="pq")
                    for c in range(NCH):
                        nc.tensor.matmul(
                            out=pp[:, :G + 4],
                            lhsT=emb_bf[:, c, hc * 128:(hc + 1) * 128],
                            rhs=maskT[:, c, :],
                            start=(c == 0), stop=(c == NCH - 1))
                    nc.vector.tensor_copy(out=pooled[:, hc, :], in_=pp[:, :G + 1])

                # ---- q_graph^T row: qg[1, H] = mean_col.T @ Wg ----
                qg_ps = ps_pq.tile([1, H], F32, name="qg", tag="pq")
                for kc in range(2):
                    nc.tensor.matmul(
                        out=qg_ps[:, :],
                        lhsT=pooled[:, kc, G:G + 1],
                        rhs=wt["wg"][:, kc, :],
                        start=(kc == 0), stop=(kc == 1))
                qg_row = tiny.tile([1, H], F32R, name="qg_row")
                nc.vector.tensor_copy(out=qg_row[:], in_=qg_ps[:, :])

                # ---- fq^T = q_lf + q_vis + qg (rank-1 broadcast matmul) ----
                fq = fq_p.tile([128, 2, G], F32R, name="fq")
                for hc in range(2):
                    qp = ps_pq.tile([128, G], F32, name="qp", tag="pq")
                    mms = []
                    for kc in range(2):
                        mms.append((lastT[:, kc, :], wt["wlf"][:, kc, hc * 128:(hc + 1) * 128]))
                    for kc in range(2):
                        mms.append((pooled[:, kc, 0:G], wt["wv"][:, kc, hc * 128:(hc + 1) * 128]))
                    # qg broadcast over g: rank-1 matmul, K=1
                    mms.append((ones_row[:, :], qg_row[:1, hc * 128:(hc + 1) * 128]))
                    for i, (xap, wap) in enumerate(mms):
                        nc.tensor.matmul(
                            out=qp[:, :G], lhsT=wap, rhs=xap,
                            start=(i == 0), stop=(i == len(mms) - 1))
                    nc.vector.tensor_copy(out=fq[:, hc, :], in_=qp[:, :G])

                return dict(fq=fq, embT=st["embT"], dmask=dmask)

            def tail(b, st):
                fq, embT, dmask = st["fq"], st["embT"], st["dmask"]
                obuf = obp.tile([GC, NGC, N], BF16, name="obuf")
                # ---- score + softmax per g-chunk ----
                for gc in range(NGC):
                    # one PSUM tile per 500-col half: a matmul output must stay
                    # inside a single 2KB PSUM bank
                    sc = [ps_sc.tile([GC, 500], F32, name="sc", tag="sc")
                          for _ in range(2)]
                    for nh in range(2):
                        for kc in range(2):
                            nc.tensor.matmul(
                                out=sc[nh][:, :],
                                lhsT=fq[:, kc, gc * GC:(gc + 1) * GC],
                                rhs=embT[:, kc, nh * 500:(nh + 1) * 500],
                                start=(kc == 0), stop=(kc == 1))
                    # z = score + (mask - dist/sqrt2); tanh saturation applies
                    # the -2^26 mask (visited -> clip exactly -10)
                    z = sm.tile([GC, N], F32, name="z")
                    for nh in range(2):
                        nc.vector.tensor_tensor(
                            out=z[:, nh * 500:(nh + 1) * 500],
                            in0=sc[nh][:, :],
                            in1=dmask[:, gc, nh * 500:(nh + 1) * 500], op=Alu.add)
                    t_ = sm.tile([GC, N], F32, name="t")
                    nc.scalar.activation(out=t_[:], in_=z[:], func=Act.Tanh, scale=1.0)
                    # 10*tanh bounds scores to [-10, 10]: exp never overflows,
                    # so no row-max stabilization is needed
                    e = z                                  # write exp in place
                    s = tiny.tile([GC, 1], F32, name="s")
                    nc.scalar.activation(
                        out=e[:], in_=t_[:], func=Act.Exp,
                        scale=TANH_CLIP, accum_out=s[:, :1])
                    r = tiny.tile([GC, 1], F32, name="r")
                    nc.vector.reciprocal(out=r[:], in_=s[:, :1])
                    nc.vector.tensor_tensor(
                        out=obuf[:, gc, :], in0=e[:],
                        in1=r[:, 0:1].to_broadcast([GC, N]), op=Alu.mult)
                # single contiguous store (8KB per partition, bf16) on the
                # scalar HWDGE ring (separate from the sync load ring)
                nc.scalar.dma_start(
                    out=out_e[b].rearrange("(p c) n -> p c n", c=NGC), in_=obuf[:])

            # software pipeline: DMAs for b+1 issue before tail(b); next-batch
            # compute is emitted after tail(b) so tail ops aren't queued behind
            # it on shared engines
            stL = head_load(0)
            stC = head_compute(0, stL)
            for b in range(nb):
                stL_next = head_load(b + 1) if b + 1 < nb else None
                tail(b, stC)
                stC = head_compute(b + 1, stL_next) if stL_next else None

    _split_excess_waits(nc)
    return nc


_NC_CACHE = {}


def _get_nc(nb=NB):
    if nb not in _NC_CACHE:
        _NC_CACHE[nb] = build_nc(nb)
    return _NC_CACHE[nb]


def _prep_weights(Wq_graph, Wq_first, Wq_last, W_visited):
    Wq_graph = np.asarray(Wq_graph, np.float32)
    Wq_first = np.asarray(Wq_first, np.float32)
    Wq_last = np.asarray(Wq_last, np.float32)
    W_visited = np.asarray(W_visited, np.float32)
    s_h = np.float32(INV_SQRT_H)
    wlf = _r12((Wq_last + Wq_first).T * s_h)
    # maskprob is -2^26 * visited; fold the sign and scale into W_visited
    wv = _r12(W_visited.T * (-s_h / np.float32(N * (-MASK_NEG))))
    wg = _r12(Wq_graph.T * (s_h / np.float32(N)))
    out = {}
    for nm, w in (("wlf", wlf), ("wv", wv), ("wg", wg)):
        out[nm] = np.ascontiguousarray(
            w.reshape(2, 128, H).transpose(1, 0, 2))
    return out


def _make_in_maps(embeddings, dists, last_node, group_ninf_mask,
                  Wq_graph, Wq_first, Wq_last, W_visited):
    emb = np.ascontiguousarray(np.asarray(embeddings), np.float32)
    # embT[b, p, kc, n] = r12(emb[b, n, kc*128 + p])
    embt = _r12(np.ascontiguousarray(
        emb.transpose(0, 2, 1).reshape(B, 2, 128, N).transpose(0, 2, 1, 3)))
    embbf = emb.astype(ml_dtypes.bfloat16)
    dist16 = np.asarray(dists).astype(np.float16)
    maskp = np.maximum(np.asarray(group_ninf_mask, np.float32),
                       np.float32(MASK_NEG)).astype(ml_dtypes.bfloat16)
    ln = np.asarray(last_node).astype(np.int32)
    w = _prep_weights(Wq_graph, Wq_first, Wq_last, W_visited)
    in_maps = []
    for c in range(NCORES):
        sl = slice(c * NB, (c + 1) * NB)
        idx_host = np.ascontiguousarray(
            ln[sl].reshape(NB, GC, NGC).transpose(1, 0, 2).reshape(GC, NB * NGC))
        m = dict(emb=emb[sl], embt=embt[sl], embbf=embbf[sl],
                 dists=dist16[sl], last_node=idx_host,
                 mask=np.ascontiguousarray(maskp[sl]))
        m.update(w)
        in_maps.append(m)
    return in_maps


def kernel(embeddings, dists, last_node, group_ninf_mask,
           Wq_graph, Wq_first, Wq_last, W_visited, **_ignored):
    in_maps = _make_in_maps(embeddings, dists, last_node, group_ninf_mask,
                            Wq_graph, Wq_first, Wq_last, W_visited)
    nc = _get_nc(NB)
    res = run_bass_kernel_spmd(nc, in_maps, list(range(NCORES)))
    out = np.concatenate([np.asarray(res.results[c]["out"]) for c in range(NCORES)],
                         axis=0)
    return out.astype(np.float32)


if __name__ == "__main__":
    # quick smoke test with random data
    rng = np.random.default_rng(0)
    emb = rng.standard_normal((B, N, H), dtype=np.float32)
    d = rng.random((B, N, N), dtype=np.float32)
    lnod = rng.integers(0, N, (B, G)).astype(np.int32)
    visited = rng.random((B, G, N)) < 0.3
    mask = np.where(visited, -np.inf, 0.0).astype(np.float32)
    s = 1.0 / np.sqrt(H)
    ws = [rng.standard_normal((H, H), dtype=np.float32) * s for _ in range(4)]
    o = kernel(emb, d, lnod, mask, *ws)
    print("out", o.shape, o.dtype, o.sum())
